# revision 71
# baseline (speedup 1.0000x reference)
"""Trainium2 Bass kernel for nn_Attention_82403242541756.

Reference semantics (with the dim-0 chunk bug):
  qkv = inputs @ W_qkv + b_qkv                  # [3, 2048, 3072]
  q, k, v = split(qkv, 3, axis=0)               # batch split! q=batch0, k=batch1, v=batch2
  each chunk [1, 2048, 3072] flat-reinterpreted to (3, 16, 2048, 64) = 48 "heads"
  scoresT softmax (no max needed; |scores| < 2.2), ctx, flat-reinterpret, @ W_out + b_out

Sharding (zero communication): core c takes seq rows [256c, 256c+256) of all 3
batch items. Head g's flat chunk [g*131072, (g+1)*131072) of a batch's [2048*3072]
QKV output aligns exactly with rows [256c, 256c+256) for g in [6c, 6c+6), and the
output-side reinterpret puts head g at rows [128g, 128g+128) of the flattened
[6144, 1024] context, i.e. rows [768c, 768c+768) of the final output per core.

v4 layout/schedule notes:
  - ctx matmul is oriented [s-partitions, d-free] (lhsT = exp chunk, rhs = v
    with a ones column): ap per matmul is 65 instead of 512, halving ctx PE
    time, and the softmax denominator lands in a per-partition column.
  - the exp activation's output AP parity-interleaves each 128-col block
    (col = 64*(s%2) + (s%128)//2) so ctx psum partitions come out as
    (t%2, s//2); per head the normalized ctx is then routed DRAM->XBAR
    transpose into ctxn2[p=64*(t%2)+d, sc, j], giving the out-projection a
    full 128-deep contraction (8 accumulation steps instead of 16).
  - PSUM: "sc" ring (3 x 2 banks) carries scores and out-proj psums; "ps"
    ring (2 x 1 bank) carries QKV psums and ctx chunks. 8 banks total.
  - all DMA consumer/producer pairs on DRAM scratch share one queue (SP):
    cross-queue DMA->DMA ordering proved racy on real HW.
  - engines execute in-order, so emission is software-pipelined: heads 0-1's
    scores/exps interleave with the QKV units (b2 early so v/ctx(0) are
    ready; 4 m1 units ride the post-ctx(0) lockstep stretch, 2 more ride
    round 2); round lf = frontend(lf) lockstep + ctx(lf-1) + outproj(lf-2);
    head 5 runs hh-major so ctx(5, 0..7) + its gather fit in round 5, and
    the tail splits outproj(5) by sc-half to shorten the final chain.
"""

import sys

sys.path.insert(0, "/opt/trn_rl_repo")

import numpy as np
import ml_dtypes

from concourse import bacc, bass, mybir, tile
from concourse.bass_utils import run_bass_kernel_spmd

BF16 = mybir.dt.bfloat16
F32 = mybir.dt.float32
AF = mybir.ActivationFunctionType
ALU = mybir.AluOpType

P = 128
N_CORES = 8
SEQ = 2048
H = 1024
HEADS_PER_CORE = 6
ROWS = 256  # seq rows per core
SCALE = float(H) ** -0.5  # 1/32, folded into the exp activation

_NC_CACHE = {}


def _build():
    nc = bacc.Bacc()

    xt_e = nc.declare_dram_parameter("xt", [P, 8, 768], BF16, isOutput=False)
    wq_e = nc.declare_dram_parameter("wq", [P, 8, 3072], BF16, isOutput=False)
    bq_e = nc.declare_dram_parameter("bq", [P, 3072], F32, isOutput=False)
    wo_e = nc.declare_dram_parameter("wo", [P, 8, 1024], BF16, isOutput=False)
    bo_e = nc.declare_dram_parameter("bo", [P, 8], F32, isOutput=False)
    out_e = nc.declare_dram_parameter("outt", [1024, 768], F32, isOutput=True)

    with tile.TileContext(nc) as tc:
        with (
            tc.tile_pool(name="dram", bufs=1, space="DRAM") as dp,
            tc.tile_pool(name="qk", bufs=4) as qkp,
            tc.tile_pool(name="vex", bufs=2) as vxp,
            tc.tile_pool(name="scps", bufs=3, space="PSUM") as scps_p,
            tc.tile_pool(name="psp", bufs=2, space="PSUM") as psp,
            tc.tile_pool(name="expp", bufs=3) as expp,
            tc.tile_pool(name="rs", bufs=2) as rsp,
            tc.tile_pool(name="stg", bufs=3) as stgp,
        ):
            # Padded to 128 cols so the bf16 XBAR DMA-transpose readback is legal.
            yq = dp.tile([12288, 128], BF16)
            yk = dp.tile([12288, 128], BF16)
            yv = dp.tile([12288, 64], BF16)
            yq_v = yq.rearrange("(r j) d -> r j d", j=48)
            yk_v = yk.rearrange("(r j) d -> r j d", j=48)
            yv_v = yv.rearrange("(r j) d -> r (j d)", j=48)

            import contextlib

            es1 = contextlib.ExitStack()
            es2 = contextlib.ExitStack()
            # es2's pools are created FIRST so es1 (closed earlier) pops in
            # proper stack order
            w1b = es2.enter_context(tc.tile_pool(name="w1b", bufs=1, side="right"))
            ybp = es2.enter_context(tc.tile_pool(name="yb", bufs=4, side="right"))
            w1a = es1.enter_context(tc.tile_pool(name="w1a", bufs=1))

            # phase-1 staging is split so the m1-column half (w1b) can stay
            # alive through round 2, where the last 6 QKV units run in PE
            # slack under the ACT-bound exp stream.
            rr3 = [nc.sync, nc.scalar, nc.gpsimd]
            xt_a = w1a.tile([P, 8, 384], BF16)  # m=0 cols of each b
            xt_b = w1b.tile([P, 8, 384], BF16)  # m=1 cols
            xt_v = xt_e.rearrange("p k (b m r) -> p k b m r", b=3, m=2)
            for kk in range(4):
                ks = slice(2 * kk, 2 * (kk + 1))
                rr3[kk % 3].dma_start(
                    xt_a[:, ks, :].rearrange("p k (b r) -> p k b r", b=3),
                    xt_v[:, ks, :, 0, :],
                )
            wq_lo = w1a.tile([P, 8, 1536], BF16)
            wq_hi = w1b.tile([P, 8, 1536], BF16)
            for k in range(8):
                rr3[(k + 1) % 3].dma_start(wq_lo[:, k, :], wq_e[:, k, 0:1536])
            # xt_b (m1 columns) is first consumed ~60us in - load it after
            # the m0-critical wq_lo stream
            for kk in range(4):
                ks = slice(2 * kk, 2 * (kk + 1))
                rr3[(kk + 1) % 3].dma_start(
                    xt_b[:, ks, :].rearrange("p k (b r) -> p k b r", b=3),
                    xt_v[:, ks, :, 1, :],
                )
            bq_lo = w1a.tile([P, 1536], F32)
            bq_hi = w1b.tile([P, 1536], F32)
            for cc in range(3):
                nc.gpsimd.dma_start(
                    bq_lo[:, 512 * cc : 512 * (cc + 1)],
                    bq_e[:, 512 * cc : 512 * (cc + 1)],
                )
                nc.gpsimd.dma_start(
                    bq_hi[:, 512 * cc : 512 * (cc + 1)],
                    bq_e[:, 1536 + 512 * cc : 1536 + 512 * (cc + 1)],
                )
            # second wq half off SP: the ybuf write stream + qT0/kT0
            # transposes are SP's critical path
            for k in range(8):
                eng = nc.scalar if k % 2 == 0 else nc.gpsimd
                eng.dma_start(wq_hi[:, k, :], wq_e[:, k, 1536:3072])
            # one-time zero of the yq/yk XBAR pad cols (sim finiteness; the
            # transposed pad partitions are never read by compute). m0 rows
            # first so qT0/kT0 aren't gated on the rest.
            z64 = w1a.tile([P, 64], BF16)
            nc.vector.memset(z64[:], 0.0)
            zrow = dp.tile([1, 64], BF16)
            nc.gpsimd.dma_start(zrow[:], z64[0:1, :])
            zsrc = zrow[0:1, :]
            for y in (yq, yk):
                nc.gpsimd.dma_start(y[0:6144, 64:128], zsrc.to_broadcast([6144, 64]))
            for y in (yq, yk):
                nc.gpsimd.dma_start(
                    y[6144:12288, 64:128], zsrc.to_broadcast([6144, 64])
                )

            def emit_qkv_unit(b, m, nb):
                ps = psp.tile([P, 512], F32, name=f"yps{b}_{m}_{nb}", tag="ps")
                xt_t = xt_a if m == 0 else xt_b
                wq_t, nb3 = (wq_lo, nb) if nb < 3 else (wq_hi, nb - 3)
                for k in range(8):
                    lhs = xt_t[:, k, 128 * b : 128 * (b + 1)]
                    nc.tensor.matmul(
                        ps[:],
                        lhsT=lhs,
                        rhs=wq_t[:, k, 512 * nb3 : 512 * (nb3 + 1)],
                        start=(k == 0),
                        stop=(k == 7),
                    )
                if b < 2:
                    # data cols only; the 64:128 XBAR pad cols of yq/yk are
                    # never read by compute (qT/kT partitions 64:128 unused),
                    # so they stay unwritten
                    ybuf = ybp.tile([P, 8, 64], BF16, tag="ybw")
                    nc.vector.tensor_tensor(
                        ybuf[:],
                        ps.rearrange("p (j d) -> p j d", d=64),
                        (bq_lo if nb < 3 else bq_hi)[
                            :, 512 * (nb % 3) : 512 * (nb % 3 + 1)
                        ].rearrange("p (j d) -> p j d", d=64),
                        ALU.add,
                    )
                    dst = (yq_v if b == 0 else yk_v)[
                        128 * m : 128 * (m + 1), 8 * nb : 8 * (nb + 1), 0:64
                    ]
                    nc.sync.dma_start(dst, ybuf[:])
                else:
                    ybuf = ybp.tile([P, 512], BF16, tag="ybn")
                    nc.vector.tensor_tensor(
                        ybuf[:],
                        ps[:],
                        (bq_lo if nb < 3 else bq_hi)[
                            :, 512 * (nb % 3) : 512 * (nb % 3 + 1)
                        ],
                        ALU.add,
                    )
                    nc.sync.dma_start(
                        yv_v[128 * m : 128 * (m + 1), 512 * nb : 512 * (nb + 1)],
                        ybuf[:],
                    )

            def emit_vx(l):
                # vx must ride the SAME queue (SP) as the yv writes: DMA->DMA
                # ordering across queues proved racy on HW (heads whose vx
                # loads land close to the b2 writes came out corrupted)
                vx = vxp.tile([P, 16, 65], BF16, name=f"vx{l}", tag="vx")
                nc.vector.memset(vx[:, :, 64:65], 1.0)
                nc.sync.dma_start(
                    vx[:, :, 0:64],
                    yv[SEQ * l : SEQ * (l + 1), :].rearrange("(so p) d -> p so d", p=P),
                )
                return vx

            def emit_qT(l):
                # SAME queue (SP) as the yq/yk writes - cross-queue DMA->DMA
                # ordering is racy on HW (see vx note)
                qT = qkp.tile([P, SEQ], BF16, tag="qk", name=f"qT{l}")
                nc.sync.dma_start(qT[:], yq[SEQ * l : SEQ * (l + 1), :], transpose=True)
                return qT

            def emit_kT(l):
                kT = qkp.tile([P, SEQ], BF16, tag="qk", name=f"kT{l}")
                nc.sync.dma_start(kT[:], yk[SEQ * l : SEQ * (l + 1), :], transpose=True)
                return kT

            def emit_qkT(l):
                return emit_qT(l), emit_kT(l)

            fe = {}  # head -> (qT, kT, expTs)

            def emit_frontend_alloc(l):
                qT, kT = emit_qkT(l)
                expTs = [
                    expp.tile([P, 8, SEQ], BF16, tag="expT", name=f"expT{l}_{th}")
                    for th in range(2)
                ]
                fe[l] = (qT, kT, expTs)

            def emit_score_exp(l, tt, hh):
                qT, kT, expTs = fe[l]
                th, t8 = tt // 8, tt % 8
                sc = scps_p.tile([P, 1024], F32, name=f"sc{l}_{tt}_{hh}", tag="sc")
                for s2 in range(2):
                    s0 = 1024 * hh + 512 * s2
                    nc.tensor.matmul(
                        sc[:, 512 * s2 : 512 * (s2 + 1)],
                        lhsT=kT[0:64, 128 * tt : 128 * (tt + 1)],
                        rhs=qT[0:64, s0 : s0 + 512],
                        start=True,
                        stop=True,
                    )
                # out AP parity-interleaves each 128-col block (col = 64*(s%2)
                # + (s%128)//2) so ctx lhsT can be a contiguous 1-free-dim
                # slice (HW matmul requires that for the stationary operand)
                nc.scalar.activation(
                    expTs[th][:, t8, 1024 * hh : 1024 * (hh + 1)].rearrange(
                        "p (sb t j) -> p sb j t", t=2, j=64
                    ),
                    sc[:],
                    AF.Exp,
                    scale=SCALE,
                )

            def unit(l, i):
                if l == 5:  # hh-major: first 8 ctx chunks ready mid-round
                    return (i % 16, i // 16)
                return (i // 2, i % 2)

            # ---------------- backend ----------------
            bk = {}  # head -> vx
            stage_all = {}  # head -> [128 (t%2,s//2), 16 sc, 64 d] normalized ctx

            def emit_ctx_chunk(l, scb):
                vx = bk[l]
                _, _, expTs = fe[l]
                if l not in stage_all:
                    stage_all[l] = stgp.tile(
                        [P, 16, 64], BF16, name=f"stga{l}", tag="stga"
                    )
                ctxps = psp.tile([P, 512], F32, name=f"ctxps{l}_{scb}", tag="ps")
                for tt in range(16):
                    th, t8 = tt // 8, tt % 8
                    # cols are already (t%2, s//2)-interleaved by the exp
                    # activation's scatter AP
                    lhsT = expTs[th][:, t8, 128 * scb : 128 * (scb + 1)]
                    nc.tensor.matmul(
                        ctxps[:, 0:65],
                        lhsT=lhsT,
                        rhs=vx[:, tt, :],
                        start=(tt == 0),
                        stop=(tt == 15),
                    )
                rr = rsp.tile([P, 1], F32, tag="rr")
                nc.vector.reciprocal(rr[:], ctxps[:, 64:65])
                nc.vector.tensor_scalar(
                    stage_all[l][:, scb, :], ctxps[:, 0:64], rr[:], None, ALU.mult
                )

            def emit_ctx_gather(l, half=None, eng=None):
                eng = eng or nc.sync
                # partition-shift the two parity halves into DRAM rows
                # (sc, j) x cols (t%2, d), then XBAR-transpose straight into
                # the 128-deep-contraction ctxn2 layout
                sa = stage_all[l]
                if l not in ctxd_tiles:
                    ctxd_tiles[l] = dp.tile([1024, 128], BF16, name=f"ctxd{l}")
                cd = ctxd_tiles[l]
                s0, s1 = (0, 16) if half is None else (8 * half, 8 * (half + 1))
                v = cd.rearrange("(sc j) c -> j sc c", j=64)
                eng.dma_start(v[:, s0:s1, 0:64], sa[0:64, s0:s1, :])
                eng.dma_start(v[:, s0:s1, 64:128], sa[64:128, s0:s1, :])
                dst = (
                    ctxn5b[:, :, :]
                    if (l == 5 and half == 1)
                    else ctxn2[:, l, s0:s1, :]
                )
                eng.dma_start(
                    dst.rearrange("p s j -> p (s j)"),
                    cd[64 * s0 : 64 * s1, :],
                    transpose=True,
                )

            def emit_outproj_m(l, m, half=None, out_eng=None):
                # rides the scores psum ring - no extra banks, keeps ps parity.
                # half splits output rows by sc-half (r < 64 needs only ctxn2
                # sc 0..8), letting the last head's first half run before its
                # final ctx chunks are gathered.
                if l == 5 and half == 1:
                    rhs_v = ctxn5b.rearrange("p s (jr u) -> p u s jr", u=8)
                    rv_off = 8
                else:
                    rhs_v = ctxn2[:, l].rearrange("p s (jr u) -> p u s jr", u=8)
                    rv_off = 0
                r0, r1 = (0, 128) if half is None else (64 * half, 64 * (half + 1))
                n = r1 - r0
                ops = scps_p.tile([P, 1024], F32, name=f"op{l}_{m}_{r0}", tag="sc")
                for u in range(8):
                    nc.tensor.matmul(
                        ops[:, 0:n],
                        lhsT=wo_sb[:, u, 128 * m : 128 * (m + 1)],
                        rhs=rhs_v[:, u, r0 // 8 - rv_off : r1 // 8 - rv_off, :],
                        start=(u == 0),
                        stop=(u == 7),
                    )
                ost = ost_tiles[l]
                nc.vector.tensor_scalar(
                    ost[:, m, r0:r1], ops[:, 0:n], bo_sb[:, m : m + 1], None, ALU.add
                )
                if m == 3 and l == 5 and half == 1:
                    # early half of the very last output DMA
                    nc.sync.dma_start(
                        out_e.rearrange("(m p) r -> p m r", p=P)[
                            :, 0:4, 128 * l + r0 : 128 * l + r1
                        ],
                        ost[:, 0:4, r0:r1],
                    )
                if m == 7:
                    ms = 4 if (l == 5 and half == 1) else 0
                    (out_eng or nc.sync).dma_start(
                        out_e.rearrange("(m p) r -> p m r", p=P)[
                            :, ms:8, 128 * l + r0 : 128 * l + r1
                        ],
                        ost[:, ms:8, r0:r1],
                    )

            # ---------------- emission schedule ----------------
            # prefix: m0 blocks of b0/b1 (covers q/k of heads 0-2)
            for nb in range(6):
                emit_qkv_unit(0, 0, nb)
            for nb in range(2):
                emit_qkv_unit(1, 0, nb)
            # qT0 slots into SP's idle gap between yk writes (its yq inputs
            # are already complete), so it doesn't delay the kT0 chain
            qT0 = emit_qT(0)
            for nb in range(2, 6):
                emit_qkv_unit(1, 0, nb)
            kT0 = emit_kT(0)
            expTs0 = [
                expp.tile([P, 8, SEQ], BF16, tag="expT", name=f"expT0_{th}")
                for th in range(2)
            ]
            fe[0] = (qT0, kT0, expTs0)
            emit_frontend_alloc(1)
            # interleave remaining QKV (b2 first -> v/ctx(0) early) with
            # heads 0-1 score units (2 per QKV unit)
            qkv_rest = [(2, m, nb) for m in range(2) for nb in range(6)] + [
                (b, 1, nb) for b in range(2) for nb in range(3)
            ]
            si = 0
            for qi, (b, m, nb) in enumerate(qkv_rest):
                emit_qkv_unit(b, m, nb)
                for _ in range(2 if qi % 3 == 0 else 3):
                    l, i = divmod(si, 32)
                    emit_score_exp(l, *unit(l, i))
                    si += 1
                if (b, m, nb) == (2, 1, 5):
                    bk[0] = emit_vx(0)
            es1.close()  # release the m0-half staging

            with (
                tc.tile_pool(name="w2", bufs=1) as w2p,
                tc.tile_pool(name="osb", bufs=2) as osbp,
            ):
                wo_sb = w2p.tile([P, 8, 1024], BF16)
                nc.sync.dma_start(wo_sb[:], wo_e[:])
                bo_sb = w2p.tile([P, 8], F32)
                nc.sync.dma_start(bo_sb[:], bo_e[:])
                # merged transposed-context, 128-deep-contraction layout:
                # ctxn2[p = 64*(t%2) + d, l, sc, j'] with s = 128*sc + 2*j' + t%2
                ctxn2 = w2p.tile([P, HEADS_PER_CORE, 16, 64], BF16)
                # head 5's sc 8..16 half lives in its own tile so the tail
                # gather's transpose doesn't false-WAR against op5A's reads
                ctxn5b = w2p.tile([P, 8, 64], BF16)
                ost_tiles = {}
                ctxd_tiles = {}

                # phase-1 coda: ctx(0) runs compactly (ACT still owes the
                # last ~8us of head-0/1 exps, covering it), then head-1's
                # remaining units lockstep with outproj(0) riding along.
                bk[1] = emit_vx(1)
                emit_frontend_alloc(2)
                for c in range(16):
                    emit_ctx_chunk(0, c)
                emit_ctx_gather(0)
                ost_tiles[0] = osbp.tile([P, 8, 128], F32, name="ost0", tag="ost")
                for j in range(16):
                    l, i = divmod(si, 32)
                    emit_score_exp(l, *unit(l, i))
                    si += 1
                    if j == 2:
                        emit_qkv_unit(0, 1, 3)
                    if j == 5:
                        emit_qkv_unit(1, 1, 3)
                    if j == 8:
                        emit_qkv_unit(0, 1, 4)
                    if j == 11:
                        emit_qkv_unit(0, 1, 5)
                    if j >= 8:
                        emit_outproj_m(0, j - 8)
                assert si == 64

                # steady rounds: frontend(lf) + ctx(lf-1) + outproj(lf-2)
                qkv_round2 = [(1, 1, 4), (1, 1, 5)]
                for lf in range(2, HEADS_PER_CORE):
                    bk[lf] = emit_vx(lf)
                    lo = lf - 2
                    if lo >= 1:  # op(0) already ran in the coda
                        ost_tiles[lo] = osbp.tile(
                            [P, 8, 128], F32, name=f"ost{lo}", tag="ost"
                        )
                    for i in range(32):
                        emit_score_exp(lf, *unit(lf, i))
                        if lf < 5:
                            if i % 2 == 0:
                                emit_ctx_chunk(lf - 1, i // 2)
                            if lf == 2 and i % 16 == 1:
                                emit_qkv_unit(*qkv_round2[i // 16])
                            if lo >= 1 and i % 4 == 1:
                                emit_outproj_m(lo, i // 4)

                        else:
                            # round 5 is hh-major, so th1 exps begin at unit 8
                            # and their expT-slot WAR needs ctx(4) chunks done
                            # at 1/iteration pace; op(3) + ctx(5, 0..7) ride
                            # the lighter second half
                            if i < 16:
                                emit_ctx_chunk(4, i)
                            else:
                                if i == 16:
                                    emit_ctx_gather(4)
                                if i % 2 == 0:
                                    emit_outproj_m(lo, (i - 16) // 2)
                                elif i >= 17:
                                    emit_ctx_chunk(5, (i - 17) // 2)
                    if lf < 5:
                        emit_ctx_gather(lf - 1)
                    else:
                        emit_ctx_gather(5, half=0)
                    if lf + 1 < HEADS_PER_CORE:
                        # prefetch at round END: head lf+1's qkT needs the m1
                        # rows, whose last QKV units run inside round 2
                        emit_frontend_alloc(lf + 1)
                    if lf == 2:
                        es2.close()  # QKV fully done; release the m1 staging

                # tail: ctx(5, 8..15) interleaved with outproj(5) first-half
                # (needs only the sc 0..7 gather done at round-5 end) and
                # outproj(4); then the second-half gather and outproj(5B)
                ost_tiles[4] = osbp.tile([P, 8, 128], F32, name="ost4", tag="ost")
                ost_tiles[5] = osbp.tile([P, 8, 128], F32, name="ost5", tag="ost")
                for c in range(8, 16):
                    emit_ctx_chunk(5, c)
                    # outt-A on the post-exp-idle ACT queue so SP's gather
                    # transpose isn't queue-blocked behind it
                    emit_outproj_m(5, c - 8, half=0, out_eng=nc.scalar)
                emit_ctx_gather(5, half=1)
                # keep PE at full clock through the gather-transpose wait so
                # outproj(5B) doesn't run at the mid p-state
                wps2 = scps_p.tile([P, 1024], F32, name="wps2", tag="sc")
                for _ in range(4):
                    nc.tensor.matmul(
                        wps2[:, 0:128],
                        lhsT=wo_sb[:, 0, 0:128],
                        rhs=wo_sb[:, 0, 0:128],
                        start=True,
                        stop=True,
                    )
                for m in range(8):
                    emit_outproj_m(4, m)
                for m in range(8):
                    emit_outproj_m(5, m, half=1)

    nc.finalize()
    return nc


def _get_nc():
    if "nc" not in _NC_CACHE:
        _NC_CACHE["nc"] = _build()
    return _NC_CACHE["nc"]


def kernel(inputs, W_qkv, b_qkv, W_out, b_out, _trace=False, _trace_kwargs=None):
    bf = ml_dtypes.bfloat16
    x = np.asarray(inputs, dtype=np.float32)
    Wq = np.asarray(W_qkv, dtype=np.float32)
    bq = np.asarray(b_qkv, dtype=np.float32)
    Wo = np.asarray(W_out, dtype=np.float32)
    bo = np.asarray(b_out, dtype=np.float32)

    wq_s = np.ascontiguousarray(Wq.reshape(8, P, 3072).transpose(1, 0, 2)).astype(bf)
    # wo[p = 64*tp + d, u, o] = Wo[f = 128*u + 64*tp + d, o]
    wo_s = np.ascontiguousarray(
        Wo.reshape(8, 2, 64, 1024).transpose(1, 2, 0, 3).reshape(P, 8, 1024)
    ).astype(bf)
    bq_s = np.ascontiguousarray(np.broadcast_to(bq[None, :], (P, 3072))).astype(
        np.float32
    )
    bo_s = np.ascontiguousarray(bo.reshape(8, P).T).astype(np.float32)

    in_maps = []
    for c in range(N_CORES):
        xc = x[:, ROWS * c : ROWS * (c + 1), :]  # [3, 256, 1024]
        xt = (
            xc.transpose(2, 0, 1)
            .reshape(1024, 768)
            .reshape(8, P, 768)
            .transpose(1, 0, 2)
        )
        in_maps.append(
            {
                "xt": np.ascontiguousarray(xt).astype(bf),
                "wq": wq_s,
                "bq": bq_s,
                "wo": wo_s,
                "bo": bo_s,
            }
        )

    nc = _get_nc()
    kw = {}
    if _trace:
        kw["trace"] = True
        if _trace_kwargs:
            kw.update(_trace_kwargs)
    res = run_bass_kernel_spmd(nc, in_maps, core_ids=list(range(N_CORES)), **kw)
    outs = res.results

    out = np.empty((6144, 1024), dtype=np.float32)
    for c in range(N_CORES):
        out[768 * c : 768 * (c + 1), :] = np.asarray(
            outs[c]["outt"], dtype=np.float32
        ).T
    if _trace:
        kernel.last_result = res
    return out.reshape(3, SEQ, H)


# revision 74
# speedup vs baseline: 1.0036x; 1.0036x over previous
"""Trainium2 Bass kernel for nn_Attention_82403242541756.

Reference semantics (with the dim-0 chunk bug):
  qkv = inputs @ W_qkv + b_qkv                  # [3, 2048, 3072]
  q, k, v = split(qkv, 3, axis=0)               # batch split! q=batch0, k=batch1, v=batch2
  each chunk [1, 2048, 3072] flat-reinterpreted to (3, 16, 2048, 64) = 48 "heads"
  scoresT softmax (no max needed; |scores| < 2.2), ctx, flat-reinterpret, @ W_out + b_out

Sharding (zero communication): core c takes seq rows [256c, 256c+256) of all 3
batch items. Head g's flat chunk [g*131072, (g+1)*131072) of a batch's [2048*3072]
QKV output aligns exactly with rows [256c, 256c+256) for g in [6c, 6c+6), and the
output-side reinterpret puts head g at rows [128g, 128g+128) of the flattened
[6144, 1024] context, i.e. rows [768c, 768c+768) of the final output per core.

v4 layout/schedule notes:
  - ctx matmul is oriented [s-partitions, d-free] (lhsT = exp chunk, rhs = v
    with a ones column): ap per matmul is 65 instead of 512, halving ctx PE
    time, and the softmax denominator lands in a per-partition column.
  - the exp activation's output AP parity-interleaves each 128-col block
    (col = 64*(s%2) + (s%128)//2) so ctx psum partitions come out as
    (t%2, s//2); per head the normalized ctx is then routed DRAM->XBAR
    transpose into ctxn2[p=64*(t%2)+d, sc, j], giving the out-projection a
    full 128-deep contraction (8 accumulation steps instead of 16).
  - PSUM: "sc" ring (3 x 2 banks) carries scores and out-proj psums; "ps"
    ring (2 x 1 bank) carries QKV psums and ctx chunks. 8 banks total.
  - all DMA consumer/producer pairs on DRAM scratch share one queue (SP):
    cross-queue DMA->DMA ordering proved racy on real HW.
  - engines execute in-order, so emission is software-pipelined: heads 0-1's
    scores/exps interleave with the QKV units (b2 early so v/ctx(0) are
    ready; 4 m1 units ride the post-ctx(0) lockstep stretch, 2 more ride
    round 2); round lf = frontend(lf) lockstep + ctx(lf-1) + outproj(lf-2);
    head 5 runs hh-major so ctx(5, 0..7) + its gather fit in round 5, and
    the tail splits outproj(5) by sc-half to shorten the final chain.
"""

import sys

sys.path.insert(0, "/opt/trn_rl_repo")

import numpy as np
import ml_dtypes

from concourse import bacc, bass, mybir, tile
from concourse.bass_utils import run_bass_kernel_spmd

BF16 = mybir.dt.bfloat16
F32 = mybir.dt.float32
AF = mybir.ActivationFunctionType
ALU = mybir.AluOpType

P = 128
N_CORES = 8
SEQ = 2048
H = 1024
HEADS_PER_CORE = 6
ROWS = 256  # seq rows per core
SCALE = float(H) ** -0.5  # 1/32, folded into the exp activation

_NC_CACHE = {}


def _build():
    nc = bacc.Bacc()

    xt_e = nc.declare_dram_parameter("xt", [P, 8, 768], BF16, isOutput=False)
    wq_e = nc.declare_dram_parameter("wq", [P, 8, 3072], BF16, isOutput=False)
    bq_e = nc.declare_dram_parameter("bq", [P, 3072], F32, isOutput=False)
    wo_e = nc.declare_dram_parameter("wo", [P, 8, 1024], BF16, isOutput=False)
    bo_e = nc.declare_dram_parameter("bo", [P, 8], F32, isOutput=False)
    out_e = nc.declare_dram_parameter("outt", [1024, 768], F32, isOutput=True)

    with tile.TileContext(nc) as tc:
        with (
            tc.tile_pool(name="dram", bufs=1, space="DRAM") as dp,
            tc.tile_pool(name="qk", bufs=4) as qkp,
            tc.tile_pool(name="vex", bufs=2) as vxp,
            tc.tile_pool(name="scps", bufs=3, space="PSUM") as scps_p,
            tc.tile_pool(name="psp", bufs=2, space="PSUM") as psp,
            tc.tile_pool(name="expp", bufs=3) as expp,
            tc.tile_pool(name="rs", bufs=2) as rsp,
            tc.tile_pool(name="stg", bufs=3) as stgp,
        ):
            # Padded to 128 cols so the bf16 XBAR DMA-transpose readback is legal.
            yq = dp.tile([12288, 128], BF16)
            yk = dp.tile([12288, 128], BF16)
            yv = dp.tile([12288, 64], BF16)
            yq_v = yq.rearrange("(r j) d -> r j d", j=48)
            yk_v = yk.rearrange("(r j) d -> r j d", j=48)
            yv_v = yv.rearrange("(r j) d -> r (j d)", j=48)

            import contextlib

            es1 = contextlib.ExitStack()
            es2 = contextlib.ExitStack()
            # es2's pools are created FIRST so es1 (closed earlier) pops in
            # proper stack order
            w1b = es2.enter_context(tc.tile_pool(name="w1b", bufs=1, side="right"))
            ybp = es2.enter_context(tc.tile_pool(name="yb", bufs=4, side="right"))
            w1a = es1.enter_context(tc.tile_pool(name="w1a", bufs=1))

            # phase-1 staging is split so the m1-column half (w1b) can stay
            # alive through round 2, where the last 6 QKV units run in PE
            # slack under the ACT-bound exp stream.
            rr3 = [nc.sync, nc.scalar, nc.gpsimd]
            xt_a = w1a.tile([P, 8, 384], BF16)  # m=0 cols of each b
            xt_b = w1b.tile([P, 8, 384], BF16)  # m=1 cols
            xt_v = xt_e.rearrange("p k (b m r) -> p k b m r", b=3, m=2)
            for kk in range(4):
                ks = slice(2 * kk, 2 * (kk + 1))
                rr3[kk % 3].dma_start(
                    xt_a[:, ks, :].rearrange("p k (b r) -> p k b r", b=3),
                    xt_v[:, ks, :, 0, :],
                )
            wq_lo = w1a.tile([P, 8, 1536], BF16)
            wq_hi = w1b.tile([P, 8, 1536], BF16)
            for k in range(8):
                rr3[(k + 1) % 3].dma_start(wq_lo[:, k, :], wq_e[:, k, 0:1536])
            # xt_b (m1 columns) is first consumed ~60us in - load it after
            # the m0-critical wq_lo stream
            for kk in range(4):
                ks = slice(2 * kk, 2 * (kk + 1))
                rr3[(kk + 1) % 3].dma_start(
                    xt_b[:, ks, :].rearrange("p k (b r) -> p k b r", b=3),
                    xt_v[:, ks, :, 1, :],
                )
            bq_lo = w1a.tile([P, 1536], F32)
            bq_hi = w1b.tile([P, 1536], F32)
            for cc in range(3):
                nc.gpsimd.dma_start(
                    bq_lo[:, 512 * cc : 512 * (cc + 1)],
                    bq_e[:, 512 * cc : 512 * (cc + 1)],
                )
                nc.gpsimd.dma_start(
                    bq_hi[:, 512 * cc : 512 * (cc + 1)],
                    bq_e[:, 1536 + 512 * cc : 1536 + 512 * (cc + 1)],
                )
            # second wq half off SP: the ybuf write stream + qT0/kT0
            # transposes are SP's critical path
            for k in range(8):
                eng = nc.scalar if k % 2 == 0 else nc.gpsimd
                eng.dma_start(wq_hi[:, k, :], wq_e[:, k, 1536:3072])
            # one-time zero of the yq/yk XBAR pad cols (sim finiteness; the
            # transposed pad partitions are never read by compute). m0 rows
            # first so qT0/kT0 aren't gated on the rest.
            z64 = w1a.tile([P, 64], BF16)
            nc.vector.memset(z64[:], 0.0)
            zrow = dp.tile([1, 64], BF16)
            nc.gpsimd.dma_start(zrow[:], z64[0:1, :])
            zsrc = zrow[0:1, :]
            for y in (yq, yk):
                nc.gpsimd.dma_start(y[0:6144, 64:128], zsrc.to_broadcast([6144, 64]))
            for y in (yq, yk):
                nc.gpsimd.dma_start(
                    y[6144:12288, 64:128], zsrc.to_broadcast([6144, 64])
                )

            def emit_qkv_unit(b, m, nb):
                ps = psp.tile([P, 512], F32, name=f"yps{b}_{m}_{nb}", tag="ps")
                xt_t = xt_a if m == 0 else xt_b
                wq_t, nb3 = (wq_lo, nb) if nb < 3 else (wq_hi, nb - 3)
                for k in range(8):
                    lhs = xt_t[:, k, 128 * b : 128 * (b + 1)]
                    nc.tensor.matmul(
                        ps[:],
                        lhsT=lhs,
                        rhs=wq_t[:, k, 512 * nb3 : 512 * (nb3 + 1)],
                        start=(k == 0),
                        stop=(k == 7),
                    )
                if b < 2:
                    # data cols only; the 64:128 XBAR pad cols of yq/yk are
                    # never read by compute (qT/kT partitions 64:128 unused),
                    # so they stay unwritten
                    ybuf = ybp.tile([P, 8, 64], BF16, tag="ybw")
                    nc.vector.tensor_tensor(
                        ybuf[:],
                        ps.rearrange("p (j d) -> p j d", d=64),
                        (bq_lo if nb < 3 else bq_hi)[
                            :, 512 * (nb % 3) : 512 * (nb % 3 + 1)
                        ].rearrange("p (j d) -> p j d", d=64),
                        ALU.add,
                    )
                    dst = (yq_v if b == 0 else yk_v)[
                        128 * m : 128 * (m + 1), 8 * nb : 8 * (nb + 1), 0:64
                    ]
                    nc.sync.dma_start(dst, ybuf[:])
                else:
                    ybuf = ybp.tile([P, 512], BF16, tag="ybn")
                    nc.vector.tensor_tensor(
                        ybuf[:],
                        ps[:],
                        (bq_lo if nb < 3 else bq_hi)[
                            :, 512 * (nb % 3) : 512 * (nb % 3 + 1)
                        ],
                        ALU.add,
                    )
                    nc.sync.dma_start(
                        yv_v[128 * m : 128 * (m + 1), 512 * nb : 512 * (nb + 1)],
                        ybuf[:],
                    )

            def emit_vx(l):
                # vx must ride the SAME queue (SP) as the yv writes: DMA->DMA
                # ordering across queues proved racy on HW (heads whose vx
                # loads land close to the b2 writes came out corrupted)
                vx = vxp.tile([P, 16, 65], BF16, name=f"vx{l}", tag="vx")
                nc.vector.memset(vx[:, :, 64:65], 1.0)
                nc.sync.dma_start(
                    vx[:, :, 0:64],
                    yv[SEQ * l : SEQ * (l + 1), :].rearrange("(so p) d -> p so d", p=P),
                )
                return vx

            def emit_qT(l):
                # SAME queue (SP) as the yq/yk writes - cross-queue DMA->DMA
                # ordering is racy on HW (see vx note)
                qT = qkp.tile([P, SEQ], BF16, tag="qk", name=f"qT{l}")
                nc.sync.dma_start(qT[:], yq[SEQ * l : SEQ * (l + 1), :], transpose=True)
                return qT

            def emit_kT(l):
                kT = qkp.tile([P, SEQ], BF16, tag="qk", name=f"kT{l}")
                nc.sync.dma_start(kT[:], yk[SEQ * l : SEQ * (l + 1), :], transpose=True)
                return kT

            def emit_qkT(l):
                return emit_qT(l), emit_kT(l)

            fe = {}  # head -> (qT, kT, expTs)

            def emit_frontend_alloc(l):
                qT, kT = emit_qkT(l)
                expTs = [
                    expp.tile([P, 8, SEQ], BF16, tag="expT", name=f"expT{l}_{th}")
                    for th in range(2)
                ]
                fe[l] = (qT, kT, expTs)

            def emit_score_exp(l, tt, hh):
                qT, kT, expTs = fe[l]
                th, t8 = tt // 8, tt % 8
                sc = scps_p.tile([P, 1024], F32, name=f"sc{l}_{tt}_{hh}", tag="sc")
                for s2 in range(2):
                    s0 = 1024 * hh + 512 * s2
                    nc.tensor.matmul(
                        sc[:, 512 * s2 : 512 * (s2 + 1)],
                        lhsT=kT[0:64, 128 * tt : 128 * (tt + 1)],
                        rhs=qT[0:64, s0 : s0 + 512],
                        start=True,
                        stop=True,
                    )
                # out AP parity-interleaves each 128-col block (col = 64*(s%2)
                # + (s%128)//2) so ctx lhsT can be a contiguous 1-free-dim
                # slice (HW matmul requires that for the stationary operand)
                nc.scalar.activation(
                    expTs[th][:, t8, 1024 * hh : 1024 * (hh + 1)].rearrange(
                        "p (sb t j) -> p sb j t", t=2, j=64
                    ),
                    sc[:],
                    AF.Exp,
                    scale=SCALE,
                )

            def unit(l, i):
                if l == 5:  # hh-major: first 8 ctx chunks ready mid-round
                    return (i % 16, i // 16)
                return (i // 2, i % 2)

            # ---------------- backend ----------------
            bk = {}  # head -> vx
            stage_all = {}  # head -> [128 (t%2,s//2), 16 sc, 64 d] normalized ctx

            def emit_ctx_chunk(l, scb):
                vx = bk[l]
                _, _, expTs = fe[l]
                if l not in stage_all:
                    stage_all[l] = stgp.tile(
                        [P, 16, 64], BF16, name=f"stga{l}", tag="stga"
                    )
                ctxps = psp.tile([P, 512], F32, name=f"ctxps{l}_{scb}", tag="ps")
                for tt in range(16):
                    th, t8 = tt // 8, tt % 8
                    # cols are already (t%2, s//2)-interleaved by the exp
                    # activation's scatter AP
                    lhsT = expTs[th][:, t8, 128 * scb : 128 * (scb + 1)]
                    nc.tensor.matmul(
                        ctxps[:, 0:65],
                        lhsT=lhsT,
                        rhs=vx[:, tt, :],
                        start=(tt == 0),
                        stop=(tt == 15),
                    )
                rr = rsp.tile([P, 1], F32, tag="rr")
                nc.vector.reciprocal(rr[:], ctxps[:, 64:65])
                nc.vector.tensor_scalar(
                    stage_all[l][:, scb, :], ctxps[:, 0:64], rr[:], None, ALU.mult
                )

            def emit_ctx_gather(l, half=None, eng=None):
                eng = eng or nc.sync
                # partition-shift the two parity halves into DRAM rows
                # (sc, j) x cols (t%2, d), then XBAR-transpose straight into
                # the 128-deep-contraction ctxn2 layout
                sa = stage_all[l]
                if l not in ctxd_tiles:
                    ctxd_tiles[l] = dp.tile([1024, 128], BF16, name=f"ctxd{l}")
                cd = ctxd_tiles[l]
                s0, s1 = (0, 16) if half is None else (8 * half, 8 * (half + 1))
                v = cd.rearrange("(sc j) c -> j sc c", j=64)
                eng.dma_start(v[:, s0:s1, 0:64], sa[0:64, s0:s1, :])
                eng.dma_start(v[:, s0:s1, 64:128], sa[64:128, s0:s1, :])
                dst = (
                    ctxn5b[:, :, :]
                    if (l == 5 and half == 1)
                    else ctxn2[:, l, s0:s1, :]
                )
                eng.dma_start(
                    dst.rearrange("p s j -> p (s j)"),
                    cd[64 * s0 : 64 * s1, :],
                    transpose=True,
                )

            def emit_outproj_m(l, m, half=None, out_eng=None):
                # rides the scores psum ring - no extra banks, keeps ps parity.
                # half splits output rows by sc-half (r < 64 needs only ctxn2
                # sc 0..8), letting the last head's first half run before its
                # final ctx chunks are gathered.
                if l == 5 and half == 1:
                    rhs_v = ctxn5b.rearrange("p s (jr u) -> p u s jr", u=8)
                    rv_off = 8
                else:
                    rhs_v = ctxn2[:, l].rearrange("p s (jr u) -> p u s jr", u=8)
                    rv_off = 0
                r0, r1 = (0, 128) if half is None else (64 * half, 64 * (half + 1))
                n = r1 - r0
                ops = scps_p.tile([P, 1024], F32, name=f"op{l}_{m}_{r0}", tag="sc")
                for u in range(8):
                    nc.tensor.matmul(
                        ops[:, 0:n],
                        lhsT=wo_sb[:, u, 128 * m : 128 * (m + 1)],
                        rhs=rhs_v[:, u, r0 // 8 - rv_off : r1 // 8 - rv_off, :],
                        start=(u == 0),
                        stop=(u == 7),
                    )
                ost = ost_tiles[l]
                nc.vector.tensor_scalar(
                    ost[:, m, r0:r1], ops[:, 0:n], bo_sb[:, m : m + 1], None, ALU.add
                )
                if m == 3 and l == 5 and half == 1:
                    # early half of the very last output DMA
                    nc.sync.dma_start(
                        out_e.rearrange("(m p) r -> p m r", p=P)[
                            :, 0:4, 128 * l + r0 : 128 * l + r1
                        ],
                        ost[:, 0:4, r0:r1],
                    )
                if m == 7:
                    ms = 4 if (l == 5 and half == 1) else 0
                    (out_eng or nc.sync).dma_start(
                        out_e.rearrange("(m p) r -> p m r", p=P)[
                            :, ms:8, 128 * l + r0 : 128 * l + r1
                        ],
                        ost[:, ms:8, r0:r1],
                    )

            # ---------------- emission schedule ----------------
            # prefix: m0 blocks of b0/b1 (covers q/k of heads 0-2)
            for nb in range(6):
                emit_qkv_unit(0, 0, nb)
            for nb in range(2):
                emit_qkv_unit(1, 0, nb)
            # qT0 slots into SP's idle gap between yk writes (its yq inputs
            # are already complete), so it doesn't delay the kT0 chain
            qT0 = emit_qT(0)
            for nb in range(2, 6):
                emit_qkv_unit(1, 0, nb)
            kT0 = emit_kT(0)
            expTs0 = [
                expp.tile([P, 8, SEQ], BF16, tag="expT", name=f"expT0_{th}")
                for th in range(2)
            ]
            fe[0] = (qT0, kT0, expTs0)
            emit_frontend_alloc(1)
            # interleave remaining QKV (b2 first -> v/ctx(0) early) with
            # heads 0-1 score units (2 per QKV unit)
            # b2m1's nb 3..5 are NOT here: vx(3..5) are their only consumers
            # (deadline = round-3 end) and they read only es2-resident staging,
            # so they ride rounds 2-3 in ACT-shadow PE slack
            qkv_rest = [(2, 0, nb) for nb in range(6)] + [
                (2, 1, nb) for nb in range(3)
            ] + [(b, 1, nb) for b in range(2) for nb in range(3)]
            si = 0
            for qi, (b, m, nb) in enumerate(qkv_rest):
                emit_qkv_unit(b, m, nb)
                for _ in range(3):
                    l, i = divmod(si, 32)
                    emit_score_exp(l, *unit(l, i))
                    si += 1
                if (b, m, nb) == (2, 0, 5):
                    bk[0] = emit_vx(0)  # vx(0) reads b2m0 rows only
            for _ in range(3):
                l, i = divmod(si, 32)
                emit_score_exp(l, *unit(l, i))
                si += 1
            es1.close()  # release the m0-half staging

            with (
                tc.tile_pool(name="w2", bufs=1) as w2p,
                tc.tile_pool(name="osb", bufs=2) as osbp,
            ):
                wo_sb = w2p.tile([P, 8, 1024], BF16)
                nc.sync.dma_start(wo_sb[:], wo_e[:])
                bo_sb = w2p.tile([P, 8], F32)
                nc.sync.dma_start(bo_sb[:], bo_e[:])
                # merged transposed-context, 128-deep-contraction layout:
                # ctxn2[p = 64*(t%2) + d, l, sc, j'] with s = 128*sc + 2*j' + t%2
                ctxn2 = w2p.tile([P, HEADS_PER_CORE, 16, 64], BF16)
                # head 5's sc 8..16 half lives in its own tile so the tail
                # gather's transpose doesn't false-WAR against op5A's reads
                ctxn5b = w2p.tile([P, 8, 64], BF16)
                ost_tiles = {}
                ctxd_tiles = {}

                # phase-1 coda: ctx(0) runs compactly (ACT still owes the
                # last ~8us of head-0/1 exps, covering it), then head-1's
                # remaining units lockstep with outproj(0) riding along.
                bk[1] = emit_vx(1)
                emit_frontend_alloc(2)
                for c in range(16):
                    emit_ctx_chunk(0, c)
                emit_ctx_gather(0)
                ost_tiles[0] = osbp.tile([P, 8, 128], F32, name="ost0", tag="ost")
                for j in range(16):
                    l, i = divmod(si, 32)
                    emit_score_exp(l, *unit(l, i))
                    si += 1
                    if j == 2:
                        emit_qkv_unit(0, 1, 3)
                    if j == 5:
                        emit_qkv_unit(1, 1, 3)
                    if j == 8:
                        emit_qkv_unit(0, 1, 4)
                    if j == 11:
                        emit_qkv_unit(0, 1, 5)
                    if j >= 8:
                        emit_outproj_m(0, j - 8)
                assert si == 64

                # steady rounds: frontend(lf) + ctx(lf-1) + outproj(lf-2)
                qkv_round2 = [(1, 1, 4), (1, 1, 5), (2, 1, 3), (2, 1, 4)]
                qkv_round3 = [(2, 1, 5)]
                for lf in range(2, HEADS_PER_CORE):
                    if lf != 3:
                        bk[lf] = emit_vx(lf)
                    lo = lf - 2
                    if lo >= 1:  # op(0) already ran in the coda
                        ost_tiles[lo] = osbp.tile(
                            [P, 8, 128], F32, name=f"ost{lo}", tag="ost"
                        )
                    for i in range(32):
                        emit_score_exp(lf, *unit(lf, i))
                        if lf < 5:
                            if i % 2 == 0:
                                emit_ctx_chunk(lf - 1, i // 2)
                            if lf == 2 and i % 8 == 1:
                                emit_qkv_unit(*qkv_round2[i // 8])
                            if lf == 3 and i == 3:
                                emit_qkv_unit(*qkv_round3[0])
                            if lo >= 1 and i % 4 == 1:
                                emit_outproj_m(lo, i // 4)

                        else:
                            # round 5 is hh-major, so th1 exps begin at unit 8
                            # and their expT-slot WAR needs ctx(4) chunks done
                            # at 1/iteration pace; op(3) + ctx(5, 0..7) ride
                            # the lighter second half
                            if i < 16:
                                emit_ctx_chunk(4, i)
                            else:
                                if i == 16:
                                    emit_ctx_gather(4)
                                if i % 2 == 0:
                                    emit_outproj_m(lo, (i - 16) // 2)
                                elif i >= 17:
                                    emit_ctx_chunk(5, (i - 17) // 2)
                    if lf < 5:
                        emit_ctx_gather(lf - 1)
                    else:
                        emit_ctx_gather(5, half=0)
                    if lf == 3:
                        # vx(3) reads b2m1 rows, finished inside this round
                        bk[3] = emit_vx(3)
                    if lf + 1 < HEADS_PER_CORE:
                        # prefetch at round END: head lf+1's qkT needs the m1
                        # rows, whose last QKV units run inside round 2
                        emit_frontend_alloc(lf + 1)
                    if lf == 3:
                        es2.close()  # QKV fully done; release the m1 staging

                # tail: ctx(5, 8..15) interleaved with outproj(5) first-half
                # (needs only the sc 0..7 gather done at round-5 end) and
                # outproj(4); then the second-half gather and outproj(5B)
                ost_tiles[4] = osbp.tile([P, 8, 128], F32, name="ost4", tag="ost")
                ost_tiles[5] = osbp.tile([P, 8, 128], F32, name="ost5", tag="ost")
                for c in range(8, 16):
                    emit_ctx_chunk(5, c)
                    # outt-A on the post-exp-idle ACT queue so SP's gather
                    # transpose isn't queue-blocked behind it
                    emit_outproj_m(5, c - 8, half=0, out_eng=nc.scalar)
                emit_ctx_gather(5, half=1)
                # keep PE at full clock through the gather-transpose wait so
                # outproj(5B) doesn't run at the mid p-state
                wps2 = scps_p.tile([P, 1024], F32, name="wps2", tag="sc")
                for _ in range(4):
                    nc.tensor.matmul(
                        wps2[:, 0:128],
                        lhsT=wo_sb[:, 0, 0:128],
                        rhs=wo_sb[:, 0, 0:128],
                        start=True,
                        stop=True,
                    )
                for m in range(8):
                    emit_outproj_m(4, m)
                for m in range(8):
                    emit_outproj_m(5, m, half=1)

    nc.finalize()
    return nc


def _get_nc():
    if "nc" not in _NC_CACHE:
        _NC_CACHE["nc"] = _build()
    return _NC_CACHE["nc"]


def kernel(inputs, W_qkv, b_qkv, W_out, b_out, _trace=False, _trace_kwargs=None):
    bf = ml_dtypes.bfloat16
    x = np.asarray(inputs, dtype=np.float32)
    Wq = np.asarray(W_qkv, dtype=np.float32)
    bq = np.asarray(b_qkv, dtype=np.float32)
    Wo = np.asarray(W_out, dtype=np.float32)
    bo = np.asarray(b_out, dtype=np.float32)

    wq_s = np.ascontiguousarray(Wq.reshape(8, P, 3072).transpose(1, 0, 2)).astype(bf)
    # wo[p = 64*tp + d, u, o] = Wo[f = 128*u + 64*tp + d, o]
    wo_s = np.ascontiguousarray(
        Wo.reshape(8, 2, 64, 1024).transpose(1, 2, 0, 3).reshape(P, 8, 1024)
    ).astype(bf)
    bq_s = np.ascontiguousarray(np.broadcast_to(bq[None, :], (P, 3072))).astype(
        np.float32
    )
    bo_s = np.ascontiguousarray(bo.reshape(8, P).T).astype(np.float32)

    in_maps = []
    for c in range(N_CORES):
        xc = x[:, ROWS * c : ROWS * (c + 1), :]  # [3, 256, 1024]
        xt = (
            xc.transpose(2, 0, 1)
            .reshape(1024, 768)
            .reshape(8, P, 768)
            .transpose(1, 0, 2)
        )
        in_maps.append(
            {
                "xt": np.ascontiguousarray(xt).astype(bf),
                "wq": wq_s,
                "bq": bq_s,
                "wo": wo_s,
                "bo": bo_s,
            }
        )

    nc = _get_nc()
    kw = {}
    if _trace:
        kw["trace"] = True
        if _trace_kwargs:
            kw.update(_trace_kwargs)
    res = run_bass_kernel_spmd(nc, in_maps, core_ids=list(range(N_CORES)), **kw)
    outs = res.results

    out = np.empty((6144, 1024), dtype=np.float32)
    for c in range(N_CORES):
        out[768 * c : 768 * (c + 1), :] = np.asarray(
            outs[c]["outt"], dtype=np.float32
        ).T
    if _trace:
        kernel.last_result = res
    return out.reshape(3, SEQ, H)


# revision 77
# speedup vs baseline: 1.0041x; 1.0005x over previous
"""Trainium2 Bass kernel for nn_Attention_82403242541756.

Reference semantics (with the dim-0 chunk bug):
  qkv = inputs @ W_qkv + b_qkv                  # [3, 2048, 3072]
  q, k, v = split(qkv, 3, axis=0)               # batch split! q=batch0, k=batch1, v=batch2
  each chunk [1, 2048, 3072] flat-reinterpreted to (3, 16, 2048, 64) = 48 "heads"
  scoresT softmax (no max needed; |scores| < 2.2), ctx, flat-reinterpret, @ W_out + b_out

Sharding (zero communication): core c takes seq rows [256c, 256c+256) of all 3
batch items. Head g's flat chunk [g*131072, (g+1)*131072) of a batch's [2048*3072]
QKV output aligns exactly with rows [256c, 256c+256) for g in [6c, 6c+6), and the
output-side reinterpret puts head g at rows [128g, 128g+128) of the flattened
[6144, 1024] context, i.e. rows [768c, 768c+768) of the final output per core.

v4 layout/schedule notes:
  - ctx matmul is oriented [s-partitions, d-free] (lhsT = exp chunk, rhs = v
    with a ones column): ap per matmul is 65 instead of 512, halving ctx PE
    time, and the softmax denominator lands in a per-partition column.
  - the exp activation's output AP parity-interleaves each 128-col block
    (col = 64*(s%2) + (s%128)//2) so ctx psum partitions come out as
    (t%2, s//2); per head the normalized ctx is then routed DRAM->XBAR
    transpose into ctxn2[p=64*(t%2)+d, sc, j], giving the out-projection a
    full 128-deep contraction (8 accumulation steps instead of 16).
  - PSUM: "sc" ring (3 x 2 banks) carries scores and out-proj psums; "ps"
    ring (2 x 1 bank) carries QKV psums and ctx chunks. 8 banks total.
  - all DMA consumer/producer pairs on DRAM scratch share one queue (SP):
    cross-queue DMA->DMA ordering proved racy on real HW.
  - engines execute in-order, so emission is software-pipelined: heads 0-1's
    scores/exps interleave with the QKV units (b2 early so v/ctx(0) are
    ready; 4 m1 units ride the post-ctx(0) lockstep stretch, 2 more ride
    round 2); round lf = frontend(lf) lockstep + ctx(lf-1) + outproj(lf-2);
    head 5 runs hh-major so ctx(5, 0..7) + its gather fit in round 5, and
    the tail splits outproj(5) by sc-half to shorten the final chain.
"""

import sys

sys.path.insert(0, "/opt/trn_rl_repo")

import numpy as np
import ml_dtypes

from concourse import bacc, bass, mybir, tile
from concourse.bass_utils import run_bass_kernel_spmd

BF16 = mybir.dt.bfloat16
F32 = mybir.dt.float32
AF = mybir.ActivationFunctionType
ALU = mybir.AluOpType

P = 128
N_CORES = 8
SEQ = 2048
H = 1024
HEADS_PER_CORE = 6
ROWS = 256  # seq rows per core
SCALE = float(H) ** -0.5  # 1/32, folded into the exp activation

_NC_CACHE = {}


def _build():
    nc = bacc.Bacc()

    xt_e = nc.declare_dram_parameter("xt", [P, 8, 768], BF16, isOutput=False)
    wq_e = nc.declare_dram_parameter("wq", [P, 8, 3072], BF16, isOutput=False)
    bq_e = nc.declare_dram_parameter("bq", [P, 3072], F32, isOutput=False)
    wo_e = nc.declare_dram_parameter("wo", [P, 8, 1024], BF16, isOutput=False)
    bo_e = nc.declare_dram_parameter("bo", [P, 8], F32, isOutput=False)
    out_e = nc.declare_dram_parameter("outt", [1024, 768], F32, isOutput=True)

    with tile.TileContext(nc) as tc:
        with (
            tc.tile_pool(name="dram", bufs=1, space="DRAM") as dp,
            tc.tile_pool(name="qk", bufs=4) as qkp,
            tc.tile_pool(name="vex", bufs=2) as vxp,
            tc.tile_pool(name="scps", bufs=3, space="PSUM") as scps_p,
            tc.tile_pool(name="psp", bufs=2, space="PSUM") as psp,
            tc.tile_pool(name="expp", bufs=3) as expp,
            tc.tile_pool(name="rs", bufs=2) as rsp,
            tc.tile_pool(name="stg", bufs=3) as stgp,
        ):
            # Padded to 128 cols so the bf16 XBAR DMA-transpose readback is legal.
            yq = dp.tile([12288, 128], BF16)
            yk = dp.tile([12288, 128], BF16)
            yv = dp.tile([12288, 64], BF16)
            yq_v = yq.rearrange("(r j) d -> r j d", j=48)
            yk_v = yk.rearrange("(r j) d -> r j d", j=48)
            yv_v = yv.rearrange("(r j) d -> r (j d)", j=48)

            import contextlib

            es1 = contextlib.ExitStack()
            es2 = contextlib.ExitStack()
            # es2's pools are created FIRST so es1 (closed earlier) pops in
            # proper stack order
            w1b = es2.enter_context(tc.tile_pool(name="w1b", bufs=1, side="right"))
            ybp = es2.enter_context(tc.tile_pool(name="yb", bufs=4, side="right"))
            w1a = es1.enter_context(tc.tile_pool(name="w1a", bufs=1))

            # phase-1 staging is split so the m1-column half (w1b) can stay
            # alive through round 2, where the last 6 QKV units run in PE
            # slack under the ACT-bound exp stream.
            rr3 = [nc.sync, nc.scalar, nc.gpsimd]
            xt_a = w1a.tile([P, 8, 384], BF16)  # m=0 cols of each b
            xt_b = w1b.tile([P, 8, 384], BF16)  # m=1 cols
            xt_v = xt_e.rearrange("p k (b m r) -> p k b m r", b=3, m=2)
            for kk in range(4):
                ks = slice(2 * kk, 2 * (kk + 1))
                rr3[kk % 3].dma_start(
                    xt_a[:, ks, :].rearrange("p k (b r) -> p k b r", b=3),
                    xt_v[:, ks, :, 0, :],
                )
            wq_lo = w1a.tile([P, 8, 1536], BF16)
            wq_hi = w1b.tile([P, 8, 1536], BF16)
            for k in range(8):
                rr3[(k + 1) % 3].dma_start(wq_lo[:, k, :], wq_e[:, k, 0:1536])
            # xt_b (m1 columns) is first consumed ~60us in - load it after
            # the m0-critical wq_lo stream
            for kk in range(4):
                ks = slice(2 * kk, 2 * (kk + 1))
                rr3[(kk + 1) % 3].dma_start(
                    xt_b[:, ks, :].rearrange("p k (b r) -> p k b r", b=3),
                    xt_v[:, ks, :, 1, :],
                )
            bq_lo = w1a.tile([P, 1536], F32)
            bq_hi = w1b.tile([P, 1536], F32)
            for cc in range(3):
                nc.gpsimd.dma_start(
                    bq_lo[:, 512 * cc : 512 * (cc + 1)],
                    bq_e[:, 512 * cc : 512 * (cc + 1)],
                )
                nc.gpsimd.dma_start(
                    bq_hi[:, 512 * cc : 512 * (cc + 1)],
                    bq_e[:, 1536 + 512 * cc : 1536 + 512 * (cc + 1)],
                )
            # second wq half off SP: the ybuf write stream + qT0/kT0
            # transposes are SP's critical path
            for k in range(8):
                eng = nc.scalar if k % 2 == 0 else nc.gpsimd
                eng.dma_start(wq_hi[:, k, :], wq_e[:, k, 1536:3072])
            # one-time zero of the yq/yk XBAR pad cols (sim finiteness; the
            # transposed pad partitions are never read by compute). m0 rows
            # first so qT0/kT0 aren't gated on the rest.
            z64 = w1a.tile([P, 64], BF16)
            nc.vector.memset(z64[:], 0.0)
            zrow = dp.tile([1, 64], BF16)
            nc.gpsimd.dma_start(zrow[:], z64[0:1, :])
            zsrc = zrow[0:1, :]
            for y in (yq, yk):
                nc.gpsimd.dma_start(y[0:6144, 64:128], zsrc.to_broadcast([6144, 64]))
            for y in (yq, yk):
                nc.gpsimd.dma_start(
                    y[6144:12288, 64:128], zsrc.to_broadcast([6144, 64])
                )

            def emit_qkv_unit(b, m, nb):
                ps = psp.tile([P, 512], F32, name=f"yps{b}_{m}_{nb}", tag="ps")
                xt_t = xt_a if m == 0 else xt_b
                wq_t, nb3 = (wq_lo, nb) if nb < 3 else (wq_hi, nb - 3)
                for k in range(8):
                    lhs = xt_t[:, k, 128 * b : 128 * (b + 1)]
                    nc.tensor.matmul(
                        ps[:],
                        lhsT=lhs,
                        rhs=wq_t[:, k, 512 * nb3 : 512 * (nb3 + 1)],
                        start=(k == 0),
                        stop=(k == 7),
                    )
                if b < 2:
                    # data cols only; the 64:128 XBAR pad cols of yq/yk are
                    # never read by compute (qT/kT partitions 64:128 unused),
                    # so they stay unwritten
                    ybuf = ybp.tile([P, 8, 64], BF16, tag="ybw")
                    nc.vector.tensor_tensor(
                        ybuf[:],
                        ps.rearrange("p (j d) -> p j d", d=64),
                        (bq_lo if nb < 3 else bq_hi)[
                            :, 512 * (nb % 3) : 512 * (nb % 3 + 1)
                        ].rearrange("p (j d) -> p j d", d=64),
                        ALU.add,
                    )
                    dst = (yq_v if b == 0 else yk_v)[
                        128 * m : 128 * (m + 1), 8 * nb : 8 * (nb + 1), 0:64
                    ]
                    nc.sync.dma_start(dst, ybuf[:])
                else:
                    ybuf = ybp.tile([P, 512], BF16, tag="ybn")
                    nc.vector.tensor_tensor(
                        ybuf[:],
                        ps[:],
                        (bq_lo if nb < 3 else bq_hi)[
                            :, 512 * (nb % 3) : 512 * (nb % 3 + 1)
                        ],
                        ALU.add,
                    )
                    nc.sync.dma_start(
                        yv_v[128 * m : 128 * (m + 1), 512 * nb : 512 * (nb + 1)],
                        ybuf[:],
                    )

            def emit_vx(l):
                # vx must ride the SAME queue (SP) as the yv writes: DMA->DMA
                # ordering across queues proved racy on HW (heads whose vx
                # loads land close to the b2 writes came out corrupted)
                vx = vxp.tile([P, 16, 65], BF16, name=f"vx{l}", tag="vx")
                nc.vector.memset(vx[:, :, 64:65], 1.0)
                nc.sync.dma_start(
                    vx[:, :, 0:64],
                    yv[SEQ * l : SEQ * (l + 1), :].rearrange("(so p) d -> p so d", p=P),
                )
                return vx

            def emit_qT(l):
                # SAME queue (SP) as the yq/yk writes - cross-queue DMA->DMA
                # ordering is racy on HW (see vx note)
                qT = qkp.tile([P, SEQ], BF16, tag="qk", name=f"qT{l}")
                nc.sync.dma_start(qT[:], yq[SEQ * l : SEQ * (l + 1), :], transpose=True)
                return qT

            def emit_kT(l):
                kT = qkp.tile([P, SEQ], BF16, tag="qk", name=f"kT{l}")
                nc.sync.dma_start(kT[:], yk[SEQ * l : SEQ * (l + 1), :], transpose=True)
                return kT

            def emit_qkT(l):
                return emit_qT(l), emit_kT(l)

            fe = {}  # head -> (qT, kT, expTs)

            def emit_frontend_alloc(l):
                qT, kT = emit_qkT(l)
                expTs = [
                    expp.tile([P, 8, SEQ], BF16, tag="expT", name=f"expT{l}_{th}")
                    for th in range(2)
                ]
                fe[l] = (qT, kT, expTs)

            def emit_score_exp(l, tt, hh):
                qT, kT, expTs = fe[l]
                th, t8 = tt // 8, tt % 8
                sc = scps_p.tile([P, 1024], F32, name=f"sc{l}_{tt}_{hh}", tag="sc")
                for s2 in range(2):
                    s0 = 1024 * hh + 512 * s2
                    nc.tensor.matmul(
                        sc[:, 512 * s2 : 512 * (s2 + 1)],
                        lhsT=kT[0:64, 128 * tt : 128 * (tt + 1)],
                        rhs=qT[0:64, s0 : s0 + 512],
                        start=True,
                        stop=True,
                    )
                # out AP parity-interleaves each 128-col block (col = 64*(s%2)
                # + (s%128)//2) so ctx lhsT can be a contiguous 1-free-dim
                # slice (HW matmul requires that for the stationary operand)
                nc.scalar.activation(
                    expTs[th][:, t8, 1024 * hh : 1024 * (hh + 1)].rearrange(
                        "p (sb t j) -> p sb j t", t=2, j=64
                    ),
                    sc[:],
                    AF.Exp,
                    scale=SCALE,
                )

            def unit(l, i):
                if l == 5:  # hh-major: first 8 ctx chunks ready mid-round
                    return (i % 16, i // 16)
                return (i // 2, i % 2)

            # ---------------- backend ----------------
            bk = {}  # head -> vx
            stage_all = {}  # head -> [128 (t%2,s//2), 16 sc, 64 d] normalized ctx

            def emit_ctx_chunk(l, scb):
                vx = bk[l]
                _, _, expTs = fe[l]
                if l not in stage_all:
                    stage_all[l] = stgp.tile(
                        [P, 16, 64], BF16, name=f"stga{l}", tag="stga"
                    )
                ctxps = psp.tile([P, 512], F32, name=f"ctxps{l}_{scb}", tag="ps")
                for tt in range(16):
                    th, t8 = tt // 8, tt % 8
                    # cols are already (t%2, s//2)-interleaved by the exp
                    # activation's scatter AP
                    lhsT = expTs[th][:, t8, 128 * scb : 128 * (scb + 1)]
                    nc.tensor.matmul(
                        ctxps[:, 0:65],
                        lhsT=lhsT,
                        rhs=vx[:, tt, :],
                        start=(tt == 0),
                        stop=(tt == 15),
                    )
                rr = rsp.tile([P, 1], F32, tag="rr")
                nc.vector.reciprocal(rr[:], ctxps[:, 64:65])
                nc.vector.tensor_scalar(
                    stage_all[l][:, scb, :], ctxps[:, 0:64], rr[:], None, ALU.mult
                )

            def emit_ctx_gather(l, half=None, eng=None):
                eng = eng or nc.sync
                # partition-shift the two parity halves into DRAM rows
                # (sc, j) x cols (t%2, d), then XBAR-transpose straight into
                # the 128-deep-contraction ctxn2 layout
                sa = stage_all[l]
                if l not in ctxd_tiles:
                    ctxd_tiles[l] = dp.tile([1024, 128], BF16, name=f"ctxd{l}")
                cd = ctxd_tiles[l]
                s0, s1 = (0, 16) if half is None else (8 * half, 8 * (half + 1))
                v = cd.rearrange("(sc j) c -> j sc c", j=64)
                eng.dma_start(v[:, s0:s1, 0:64], sa[0:64, s0:s1, :])
                eng.dma_start(v[:, s0:s1, 64:128], sa[64:128, s0:s1, :])
                dst = (
                    ctxn5b[:, :, :]
                    if (l == 5 and half == 1)
                    else ctxn2[:, l, s0:s1, :]
                )
                eng.dma_start(
                    dst.rearrange("p s j -> p (s j)"),
                    cd[64 * s0 : 64 * s1, :],
                    transpose=True,
                )

            def emit_outproj_m(l, m, half=None, out_eng=None):
                # rides the scores psum ring - no extra banks, keeps ps parity.
                # half splits output rows by sc-half (r < 64 needs only ctxn2
                # sc 0..8), letting the last head's first half run before its
                # final ctx chunks are gathered.
                if l == 5 and half == 1:
                    rhs_v = ctxn5b.rearrange("p s (jr u) -> p u s jr", u=8)
                    rv_off = 8
                else:
                    rhs_v = ctxn2[:, l].rearrange("p s (jr u) -> p u s jr", u=8)
                    rv_off = 0
                r0, r1 = (0, 128) if half is None else (64 * half, 64 * (half + 1))
                n = r1 - r0
                ops = scps_p.tile([P, 1024], F32, name=f"op{l}_{m}_{r0}", tag="sc")
                for u in range(8):
                    nc.tensor.matmul(
                        ops[:, 0:n],
                        lhsT=wo_sb[:, u, 128 * m : 128 * (m + 1)],
                        rhs=rhs_v[:, u, r0 // 8 - rv_off : r1 // 8 - rv_off, :],
                        start=(u == 0),
                        stop=(u == 7),
                    )
                ost = ost_tiles[l]
                nc.vector.tensor_scalar(
                    ost[:, m, r0:r1], ops[:, 0:n], bo_sb[:, m : m + 1], None, ALU.add
                )
                if m == 3 and l == 5 and half == 1:
                    # early half of the very last output DMA
                    nc.sync.dma_start(
                        out_e.rearrange("(m p) r -> p m r", p=P)[
                            :, 0:4, 128 * l + r0 : 128 * l + r1
                        ],
                        ost[:, 0:4, r0:r1],
                    )
                if m == 7:
                    ms = 4 if (l == 5 and half == 1) else 0
                    (out_eng or nc.sync).dma_start(
                        out_e.rearrange("(m p) r -> p m r", p=P)[
                            :, ms:8, 128 * l + r0 : 128 * l + r1
                        ],
                        ost[:, ms:8, r0:r1],
                    )

            # ---------------- emission schedule ----------------
            # prefix: m0 blocks of b0/b1 (covers q/k of heads 0-2)
            for nb in range(6):
                emit_qkv_unit(0, 0, nb)
            for nb in range(2):
                emit_qkv_unit(1, 0, nb)
            # qT0 slots into SP's idle gap between yk writes (its yq inputs
            # are already complete), so it doesn't delay the kT0 chain
            qT0 = emit_qT(0)
            for nb in range(2, 6):
                emit_qkv_unit(1, 0, nb)
            kT0 = emit_kT(0)
            expTs0 = [
                expp.tile([P, 8, SEQ], BF16, tag="expT", name=f"expT0_{th}")
                for th in range(2)
            ]
            fe[0] = (qT0, kT0, expTs0)
            emit_frontend_alloc(1)
            # interleave remaining QKV (b2 first -> v/ctx(0) early) with
            # heads 0-1 score units (2 per QKV unit)
            # b2m1's nb 3..5 are NOT here: vx(3..5) are their only consumers
            # (deadline = round-3 end) and they read only es2-resident staging,
            # so they ride rounds 2-3 in ACT-shadow PE slack
            qkv_rest = [(2, 0, nb) for nb in range(6)] + [
                (2, 1, nb) for nb in range(3)
            ] + [(b, 1, nb) for b in range(2) for nb in range(3)]
            si = 0
            for qi, (b, m, nb) in enumerate(qkv_rest):
                emit_qkv_unit(b, m, nb)
                for _ in range(3):
                    l, i = divmod(si, 32)
                    emit_score_exp(l, *unit(l, i))
                    si += 1
                if (b, m, nb) == (2, 0, 5):
                    bk[0] = emit_vx(0)  # vx(0) reads b2m0 rows only
            for _ in range(3):
                l, i = divmod(si, 32)
                emit_score_exp(l, *unit(l, i))
                si += 1
            es1.close()  # release the m0-half staging

            with (
                tc.tile_pool(name="w2", bufs=1) as w2p,
                tc.tile_pool(name="osb", bufs=2) as osbp,
            ):
                wo_sb = w2p.tile([P, 8, 1024], BF16)
                nc.sync.dma_start(wo_sb[:], wo_e[:])
                bo_sb = w2p.tile([P, 8], F32)
                nc.sync.dma_start(bo_sb[:], bo_e[:])
                # merged transposed-context, 128-deep-contraction layout:
                # ctxn2[p = 64*(t%2) + d, l, sc, j'] with s = 128*sc + 2*j' + t%2
                ctxn2 = w2p.tile([P, HEADS_PER_CORE, 16, 64], BF16)
                # head 5's sc 8..16 half lives in its own tile so the tail
                # gather's transpose doesn't false-WAR against op5A's reads
                ctxn5b = w2p.tile([P, 8, 64], BF16)
                ost_tiles = {}
                ctxd_tiles = {}

                # phase-1 coda: ctx(0) runs compactly (ACT still owes the
                # last ~8us of head-0/1 exps, covering it), then head-1's
                # remaining units lockstep with outproj(0) riding along.
                bk[1] = emit_vx(1)
                emit_frontend_alloc(2)
                for c in range(16):
                    emit_ctx_chunk(0, c)
                emit_ctx_gather(0)
                ost_tiles[0] = osbp.tile([P, 8, 128], F32, name="ost0", tag="ost")
                for j in range(16):
                    l, i = divmod(si, 32)
                    emit_score_exp(l, *unit(l, i))
                    si += 1
                    if j == 2:
                        emit_qkv_unit(0, 1, 3)
                    if j == 5:
                        emit_qkv_unit(1, 1, 3)
                    if j == 8:
                        emit_qkv_unit(0, 1, 4)
                    if j == 11:
                        emit_qkv_unit(0, 1, 5)
                    if j >= 8:
                        emit_outproj_m(0, j - 8)
                assert si == 64

                # steady rounds: frontend(lf) + ctx(lf-1) + outproj(lf-2)
                qkv_round2 = [(1, 1, 4), (1, 1, 5), (2, 1, 3)]
                qkv_round3 = [(2, 1, 4), (2, 1, 5)]
                for lf in range(2, HEADS_PER_CORE):
                    if lf != 3:
                        bk[lf] = emit_vx(lf)
                    lo = lf - 2
                    if lo >= 1:  # op(0) already ran in the coda
                        ost_tiles[lo] = osbp.tile(
                            [P, 8, 128], F32, name=f"ost{lo}", tag="ost"
                        )
                    for i in range(32):
                        emit_score_exp(lf, *unit(lf, i))
                        if lf < 5:
                            if i % 2 == 0:
                                emit_ctx_chunk(lf - 1, i // 2)
                            if lf == 2 and i % 8 == 1 and i // 8 < 3:
                                emit_qkv_unit(*qkv_round2[i // 8])
                            if lf == 3 and i % 8 == 3 and i // 8 < 2:
                                emit_qkv_unit(*qkv_round3[i // 8])
                            if lo >= 1 and i % 4 == 1:
                                emit_outproj_m(lo, i // 4)

                        else:
                            # round 5 is hh-major, so th1 exps begin at unit 8
                            # and their expT-slot WAR needs ctx(4) chunks done
                            # at 1/iteration pace; op(3) + ctx(5, 0..7) ride
                            # the lighter second half
                            if i < 16:
                                emit_ctx_chunk(4, i)
                            else:
                                if i == 16:
                                    emit_ctx_gather(4)
                                if i % 2 == 0:
                                    emit_outproj_m(lo, (i - 16) // 2)
                                elif i >= 17:
                                    emit_ctx_chunk(5, (i - 17) // 2)
                    if lf < 5:
                        emit_ctx_gather(lf - 1)
                    else:
                        emit_ctx_gather(5, half=0)
                    if lf == 3:
                        # vx(3) reads b2m1 rows, finished inside this round
                        bk[3] = emit_vx(3)
                    if lf + 1 < HEADS_PER_CORE:
                        # prefetch at round END: head lf+1's qkT needs the m1
                        # rows, whose last QKV units run inside round 2
                        emit_frontend_alloc(lf + 1)
                    if lf == 3:
                        es2.close()  # QKV fully done; release the m1 staging

                # tail: ctx(5, 8..15) interleaved with outproj(5) first-half
                # (needs only the sc 0..7 gather done at round-5 end) and
                # outproj(4); then the second-half gather and outproj(5B)
                ost_tiles[4] = osbp.tile([P, 8, 128], F32, name="ost4", tag="ost")
                ost_tiles[5] = osbp.tile([P, 8, 128], F32, name="ost5", tag="ost")
                for c in range(8, 16):
                    emit_ctx_chunk(5, c)
                    # outt-A on the post-exp-idle ACT queue so SP's gather
                    # transpose isn't queue-blocked behind it
                    emit_outproj_m(5, c - 8, half=0, out_eng=nc.scalar)
                emit_ctx_gather(5, half=1)
                # keep PE at full clock through the gather-transpose wait so
                # outproj(5B) doesn't run at the mid p-state
                wps2 = scps_p.tile([P, 1024], F32, name="wps2", tag="sc")
                for _ in range(4):
                    nc.tensor.matmul(
                        wps2[:, 0:128],
                        lhsT=wo_sb[:, 0, 0:128],
                        rhs=wo_sb[:, 0, 0:128],
                        start=True,
                        stop=True,
                    )
                for m in range(8):
                    emit_outproj_m(4, m)
                for m in range(8):
                    emit_outproj_m(5, m, half=1)

    nc.finalize()
    return nc


def _get_nc():
    if "nc" not in _NC_CACHE:
        _NC_CACHE["nc"] = _build()
    return _NC_CACHE["nc"]


def kernel(inputs, W_qkv, b_qkv, W_out, b_out, _trace=False, _trace_kwargs=None):
    bf = ml_dtypes.bfloat16
    x = np.asarray(inputs, dtype=np.float32)
    Wq = np.asarray(W_qkv, dtype=np.float32)
    bq = np.asarray(b_qkv, dtype=np.float32)
    Wo = np.asarray(W_out, dtype=np.float32)
    bo = np.asarray(b_out, dtype=np.float32)

    wq_s = np.ascontiguousarray(Wq.reshape(8, P, 3072).transpose(1, 0, 2)).astype(bf)
    # wo[p = 64*tp + d, u, o] = Wo[f = 128*u + 64*tp + d, o]
    wo_s = np.ascontiguousarray(
        Wo.reshape(8, 2, 64, 1024).transpose(1, 2, 0, 3).reshape(P, 8, 1024)
    ).astype(bf)
    bq_s = np.ascontiguousarray(np.broadcast_to(bq[None, :], (P, 3072))).astype(
        np.float32
    )
    bo_s = np.ascontiguousarray(bo.reshape(8, P).T).astype(np.float32)

    in_maps = []
    for c in range(N_CORES):
        xc = x[:, ROWS * c : ROWS * (c + 1), :]  # [3, 256, 1024]
        xt = (
            xc.transpose(2, 0, 1)
            .reshape(1024, 768)
            .reshape(8, P, 768)
            .transpose(1, 0, 2)
        )
        in_maps.append(
            {
                "xt": np.ascontiguousarray(xt).astype(bf),
                "wq": wq_s,
                "bq": bq_s,
                "wo": wo_s,
                "bo": bo_s,
            }
        )

    nc = _get_nc()
    kw = {}
    if _trace:
        kw["trace"] = True
        if _trace_kwargs:
            kw.update(_trace_kwargs)
    res = run_bass_kernel_spmd(nc, in_maps, core_ids=list(range(N_CORES)), **kw)
    outs = res.results

    out = np.empty((6144, 1024), dtype=np.float32)
    for c in range(N_CORES):
        out[768 * c : 768 * (c + 1), :] = np.asarray(
            outs[c]["outt"], dtype=np.float32
        ).T
    if _trace:
        kernel.last_result = res
    return out.reshape(3, SEQ, H)
